# revision 5
# baseline (speedup 1.0000x reference)
"""Causal self-attention with ALiBi for Trainium2, sharded over 8 NeuronCores.

Problem: B=2, T=2048, C=1024, H=16 heads, D=64. y = proj(softmax(qk^T/8 + alibi) v).

Sharding (per spec hint): data-parallel on B x tensor-parallel on heads.
Core c handles batch b = c // 4 and the 4 heads [4*(c%4), 4*(c%4)+4).
Each core computes its heads' attention output and a partial projection
(contracting only its 256 columns of w_proj); the host sums the 4 partials
per batch.

Host-side prep (not device work): x is pre-transposed to xT=(C,T) per batch,
weights are pre-sliced/transposed per core so the device kernel needs no
on-chip transposes. The attention scale 1/8 is folded into wq.

Device pipeline per core (all matmuls in float32r = fp22, full PE rate):
  1. qT/kT = W^T-slices @ xT     -> (64, T) per head, feature-major ("transposed")
  2. v     = x @ Wv^T            -> (T, 256) natural, with a ones column
     appended per head (gives the softmax denominator for free).
  3. ALiBi via 2 extra contraction rows: k-side [j; 1], q-side
     [slope; -slope*i] => s_T[j,i] = q.k/8 + slope*(j-i), K=66.
  4. e_T = exp(s_T) on ACT; causal mask applied by zeroing e_T's upper
     triangle on GpSimd (affine_select) for diagonal-crossing tiles only.
  5. y_aug^T = [v | 1]^T @ e_T accumulated over Tk tiles -> rows 0:64
     unnormalized y^T, row 64 the denominator.
  6. normalize via batched reciprocal + partition-broadcast multiply.
  7. partial out = y^T.T @ wp^T-slice, DMA to DRAM.

DVE/ACT engines are partition-locked (operands must share the start
partition), so moving a head's 64 rows from psum partitions 64:128 down to
0:64 goes through a small SBUF->SBUF DMA (staging tile) instead.
"""

import math

import numpy as np

B, T, C = 2, 2048, 1024
H, D = 16, 64
HL = 4          # heads per core
N_CORES = 8
P = 128         # partitions
CS = 512        # Tq chunk (matmul moving dim)
CI = C // P     # 8 contraction chunks
TT = T // P     # 16 T tiles
NQ = T // CS    # 4 Tq chunks

_BUILT = {}


def _alibi_slopes(n_heads):
    start = 2.0 ** (-(2.0 ** (-(math.log2(n_heads) - 3))))
    return np.array([start * start**i for i in range(n_heads)], dtype=np.float32)


def _build():
    """Build + compile the (single, SPMD) Bass module. Cached per process."""
    if "nc" in _BUILT:
        return _BUILT["nc"]

    from contextlib import ExitStack

    import concourse.bacc as bacc
    import concourse.mybir as mybir
    import concourse.tile as tile

    f32 = mybir.dt.float32
    f32r = mybir.dt.float32r
    EXP = mybir.ActivationFunctionType.Exp
    GE = mybir.AluOpType.is_ge

    nc = bacc.Bacc("TRN2", target_bir_lowering=False)

    xT = nc.dram_tensor("xT", [C, T], f32, kind="ExternalInput").ap()
    wqT = nc.dram_tensor("wqT", [C, HL * D], f32, kind="ExternalInput").ap()
    wkT = nc.dram_tensor("wkT", [C, HL * D], f32, kind="ExternalInput").ap()
    wvT = nc.dram_tensor("wvT", [C, HL * D], f32, kind="ExternalInput").ap()
    wpT = nc.dram_tensor("wpT", [HL * D, C], f32, kind="ExternalInput").ap()
    kaug = nc.dram_tensor("kaug", [2, T], f32, kind="ExternalInput").ap()
    qaug = nc.dram_tensor("qaug", [HL, 2, T], f32, kind="ExternalInput").ap()
    vones = nc.dram_tensor("vones", [P, HL], f32, kind="ExternalInput").ap()
    outp = nc.dram_tensor("outp", [T, C], f32, kind="ExternalOutput").ap()

    def mm(out, lhsT, rhs, start, stop):
        nc.tensor.matmul(out, lhsT.bitcast(f32r), rhs.bitcast(f32r),
                         start=start, stop=stop)

    def r(ap):
        # walrus requires every writer of an fp32r-matmul operand to declare
        # fp32r output; the PE truncates to fp22 on read either way.
        return ap.bitcast(f32r)

    with tile.TileContext(nc) as tc, ExitStack() as ctx:
        xp = ctx.enter_context(tc.tile_pool(name="xp", bufs=1))
        wpool = ctx.enter_context(tc.tile_pool(name="wpool", bufs=1))
        vp = ctx.enter_context(tc.tile_pool(name="vp", bufs=1))
        kqp = ctx.enter_context(tc.tile_pool(name="kqp", bufs=2))
        ep = ctx.enter_context(tc.tile_pool(name="ep", bufs=6))
        yp = ctx.enter_context(tc.tile_pool(name="yp", bufs=1))
        mp = ctx.enter_context(tc.tile_pool(name="mp", bufs=2))
        op_pool = ctx.enter_context(tc.tile_pool(name="op", bufs=3))
        ps_mm = ctx.enter_context(tc.tile_pool(name="ps_mm", bufs=2, space="PSUM"))
        ps_s = ctx.enter_context(tc.tile_pool(name="ps_s", bufs=3, space="PSUM"))
        ps_y = ctx.enter_context(tc.tile_pool(name="ps_y", bufs=2, space="PSUM"))

        # ---- resident loads -------------------------------------------------
        w_sb = {}
        for nm, src in (("q", wqT), ("k", wkT), ("v", wvT)):
            for ci in range(CI):
                t = wpool.tile([P, HL * D], f32, name=f"w{nm}{ci}", tag=f"w{nm}{ci}")
                nc.sync.dma_start(r(t), r(src[ci * P:(ci + 1) * P, :]))
                w_sb[nm, ci] = t
        wp_sb = []
        for i in range(2):
            t = wpool.tile([P, C], f32, name=f"wp{i}", tag=f"wp{i}")
            nc.sync.dma_start(r(t), r(wpT[i * P:(i + 1) * P, :]))
            wp_sb.append(t)
        x_sb = []
        for ci in range(CI):
            t = xp.tile([P, T], f32, name=f"x{ci}", tag=f"x{ci}")
            nc.sync.dma_start(r(t), r(xT[ci * P:(ci + 1) * P, :]))
            x_sb.append(t)

        # ---- v = x @ Wv^T, natural (T-part, 4 heads x [64 dims | ones]) ----
        v_sb = []
        for tt in range(TT):
            psv = ps_mm.tile([P, HL * D], f32, name="psv", tag="mm")
            for ci in range(CI):
                mm(psv, x_sb[ci][:, tt * P:(tt + 1) * P], w_sb["v", ci],
                   start=ci == 0, stop=ci == CI - 1)
            vt = vp.tile([P, HL * (D + 1)], f32, name=f"v{tt}", tag=f"v{tt}")
            v3 = vt.rearrange("p (h e) -> p h e", h=HL)
            nc.sync.dma_start(r(v3[:, :, D:D + 1]), r(vones.unsqueeze(2)))
            nc.vector.tensor_copy(r(v3[:, :, 0:D]),
                                  psv.rearrange("p (h d) -> p h d", h=HL))
            v_sb.append(vt)

        yT_sb = [yp.tile([P, T], f32, name=f"yT{i}", tag=f"yT{i}") for i in range(2)]
        denom_sb = mp.tile([HL * NQ, CS], f32, name="denom", tag="denom", bufs=1)

        def attention(h):
            qa, ka = qT_a[h], kT_a[h]
            for tq in range(NQ):
                i0 = tq * CS
                nk = 4 * tq + 4
                psy = ps_y.tile([D + 1, CS], f32, name="psy", tag="y")
                for kt in range(nk):
                    pss = ps_s.tile([P, CS], f32, name="pss", tag="s")
                    mm(pss, ka[:, kt * P:(kt + 1) * P], qa[:, i0:i0 + CS],
                       start=True, stop=True)
                    et = ep.tile([P, CS], f32, name="et", tag="e")
                    nc.scalar.activation(r(et), pss, EXP)
                    dlt = kt - 4 * tq
                    if dlt >= 0:  # diagonal-crossing tile: zero j > i
                        nc.gpsimd.affine_select(
                            out=r(et), in_=r(et), compare_op=GE, fill=0.0,
                            base=-P * dlt, pattern=[[1, CS]],
                            channel_multiplier=-1)
                    mm(psy, v_sb[kt][:, h * (D + 1):(h + 1) * (D + 1)], et,
                       start=kt == 0, stop=kt == nk - 1)
                # denominator row -> denom_sb[h*NQ+tq] (partition shift by DMA)
                cb = h * NQ + tq
                dstg = mp.tile([D + 1, CS], f32, name="dstg", tag="dstg")
                nc.vector.tensor_copy(dstg[D:D + 1, :], psy[D:D + 1, :])
                nc.sync.dma_start(denom_sb[cb:cb + 1, :], dstg[D:D + 1, :])
                # raw y^T rows
                if h % 2 == 0:
                    nc.vector.tensor_copy(r(yT_sb[h // 2][0:D, i0:i0 + CS]),
                                          psy[0:D, :])
                else:
                    ystg = mp.tile([D, CS], f32, name="ystg", tag="ystg")
                    nc.vector.tensor_copy(r(ystg), psy[0:D, :])
                    nc.sync.dma_start(r(yT_sb[h // 2][D:2 * D, i0:i0 + CS]), r(ystg))

        # ---- qT/kT per head pair, then attention per head -------------------
        qT_a, kT_a = {}, {}
        for m in range(2):
            for j in range(2):
                h = 2 * m + j
                qT_a[h] = kqp.tile([D + 2, T], f32, name=f"qTa{h}", tag="qTa")
                kT_a[h] = kqp.tile([D + 2, T], f32, name=f"kTa{h}", tag="kTa")
                nc.sync.dma_start(r(qT_a[h][D:D + 2, :]), r(qaug[h]))
                nc.sync.dma_start(r(kT_a[h][D:D + 2, :]), r(kaug))
            for tq in range(NQ):
                for nm, dst in (("q", qT_a), ("k", kT_a)):
                    ps = ps_mm.tile([P, CS], f32, name=f"ps{nm}", tag="mm")
                    for ci in range(CI):
                        mm(ps, w_sb[nm, ci][:, m * P:(m + 1) * P],
                           x_sb[ci][:, tq * CS:(tq + 1) * CS],
                           start=ci == 0, stop=ci == CI - 1)
                    # even head: direct copy; odd head: stage + DMA shift
                    nc.vector.tensor_copy(
                        r(dst[2 * m][0:D, tq * CS:(tq + 1) * CS]), ps[0:D, :])
                    stg = mp.tile([P, CS], f32, name=f"stg{nm}", tag="stg")
                    nc.vector.tensor_copy(r(stg[D:P, :]), ps[D:P, :])
                    nc.sync.dma_start(
                        r(dst[2 * m + 1][0:D, tq * CS:(tq + 1) * CS]), r(stg[D:P, :]))
            attention(2 * m)
            attention(2 * m + 1)

        # ---- normalize y^T by the softmax denominator ----------------------
        recip_sb = mp.tile([HL * NQ, CS], f32, name="recip", tag="recip", bufs=1)
        nc.vector.reciprocal(recip_sb, denom_sb)
        for h in range(HL):
            for tq in range(NQ):
                cb = h * NQ + tq
                rtmp = mp.tile([1, CS], f32, name="rtmp", tag="rtmp")
                nc.sync.dma_start(rtmp, recip_sb[cb:cb + 1, :])
                rb = mp.tile([P, CS], f32, name="rb", tag="rb")
                nc.gpsimd.partition_broadcast(rb, rtmp)
                r0 = (h % 2) * D
                ys = yT_sb[h // 2][r0:r0 + D, tq * CS:(tq + 1) * CS]
                nc.vector.tensor_mul(r(ys), ys, rb[r0:r0 + D, :])

        # ---- partial projection --------------------------------------------
        for tt in range(TT):
            for nh in range(2):
                psp = ps_mm.tile([P, CS], f32, name="psp", tag="mm")
                for kc in range(2):
                    mm(psp, yT_sb[kc][:, tt * P:(tt + 1) * P],
                       wp_sb[kc][:, nh * CS:(nh + 1) * CS],
                       start=kc == 0, stop=kc == 1)
                ot = op_pool.tile([P, CS], f32, name="ot", tag="o")
                nc.vector.tensor_copy(ot, psp)
                nc.sync.dma_start(outp[tt * P:(tt + 1) * P, nh * CS:(nh + 1) * CS], ot)

    nc.compile()
    _BUILT["nc"] = nc
    return nc


def _prep_inputs(x, w_attn, w_proj):
    """Shard + lay out the full inputs for the 8 cores."""
    x = np.asarray(x, dtype=np.float32)
    w_attn = np.asarray(w_attn, dtype=np.float32)
    w_proj = np.asarray(w_proj, dtype=np.float32)

    slopes = _alibi_slopes(H)
    iota = np.arange(T, dtype=np.float32)
    kaug = np.stack([iota, np.ones(T, np.float32)])  # (2, T)
    xTs = [np.ascontiguousarray(x[b].T) for b in range(B)]

    in_maps = []
    for c in range(N_CORES):
        b, hg = divmod(c, 4)
        r0, r1 = hg * 256, (hg + 1) * 256
        qaug = np.empty((HL, 2, T), np.float32)
        for j in range(HL):
            s = slopes[hg * 4 + j]
            qaug[j, 0, :] = s
            qaug[j, 1, :] = -s * iota
        in_maps.append({
            "xT": xTs[b],
            "wqT": np.ascontiguousarray(w_attn[r0:r1, :].T) * np.float32(0.125),
            "wkT": np.ascontiguousarray(w_attn[C + r0:C + r1, :].T),
            "wvT": np.ascontiguousarray(w_attn[2 * C + r0:2 * C + r1, :].T),
            "wpT": np.ascontiguousarray(w_proj[:, r0:r1].T),
            "kaug": kaug,
            "vones": np.ones((P, HL), np.float32),
            "qaug": qaug,
        })
    return in_maps


def kernel(x, w_attn, w_proj):
    from concourse import bass_utils

    nc = _build()
    in_maps = _prep_inputs(x, w_attn, w_proj)
    res = bass_utils.run_bass_kernel_spmd(nc, in_maps, core_ids=list(range(N_CORES)))
    out = np.zeros((B, T, C), dtype=np.float32)
    for c in range(N_CORES):
        out[c // 4] += res.results[c]["outp"]
    return out


# revision 14
# speedup vs baseline: 1.0142x; 1.0142x over previous
"""Causal self-attention with ALiBi for Trainium2, sharded over 8 NeuronCores.

Problem: B=2, T=2048, C=1024, H=16 heads, D=64. y = proj(softmax(qk^T/8 + alibi) v).

Sharding (per spec hint): data-parallel on B x tensor-parallel on heads.
Core c handles batch b = c // 4 and the 4 heads [4*(c%4), 4*(c%4)+4).
Each core computes its heads' attention output and a partial projection
(contracting only its 256 columns of w_proj); the host sums the 4 partials
per batch.

Host-side prep (not device work): x is pre-transposed to xT=(C,T) per batch,
weights are pre-sliced/transposed per core so the device kernel needs no
on-chip transposes. The attention scale 1/8 is folded into wq.

Device pipeline per core (all matmuls in float32r = fp22, full PE rate):
  1. qT/kT = W^T-slices @ xT     -> (64, T) per head, feature-major ("transposed")
  2. v     = x @ Wv^T            -> (T, 256) natural, with a ones column
     appended per head (gives the softmax denominator for free).
  3. ALiBi via 2 extra contraction rows: k-side [j; 1], q-side
     [slope; -slope*i] => s_T[j,i] = q.k/8 + slope*(j-i), K=66.
  4. e_T = exp(s_T) on ACT; causal mask applied by zeroing e_T's upper
     triangle on GpSimd (affine_select) for diagonal-crossing tiles only.
  5. y_aug^T = [v | 1]^T @ e_T accumulated over Tk tiles -> rows 0:64
     unnormalized y^T, row 64 the denominator.
  6. normalize via batched reciprocal + partition-broadcast multiply.
  7. partial out = y^T.T @ wp^T-slice, DMA to DRAM.

DVE/ACT engines are partition-locked (operands must share the start
partition), so moving a head's 64 rows from psum partitions 64:128 down to
0:64 goes through a small SBUF->SBUF DMA (staging tile) instead.
"""

import math

import numpy as np

B, T, C = 2, 2048, 1024
H, D = 16, 64
HL = 4          # heads per core
N_CORES = 8
P = 128         # partitions
CS = 512        # Tq chunk (matmul moving dim)
CI = C // P     # 8 contraction chunks
TT = T // P     # 16 T tiles
NQ = T // CS    # 4 Tq chunks

_BUILT = {}


def _alibi_slopes(n_heads):
    start = 2.0 ** (-(2.0 ** (-(math.log2(n_heads) - 3))))
    return np.array([start * start**i for i in range(n_heads)], dtype=np.float32)


def _build():
    """Build + compile the (single, SPMD) Bass module. Cached per process."""
    if "nc" in _BUILT:
        return _BUILT["nc"]

    from contextlib import ExitStack

    import concourse.bacc as bacc
    import concourse.mybir as mybir
    import concourse.tile as tile

    f32 = mybir.dt.float32
    f32r = mybir.dt.float32r
    EXP = mybir.ActivationFunctionType.Exp
    GE = mybir.AluOpType.is_ge

    nc = bacc.Bacc("TRN2", target_bir_lowering=False)

    xT = nc.dram_tensor("xT", [C, T], f32, kind="ExternalInput").ap()
    wqT = nc.dram_tensor("wqT", [C, HL * D], f32, kind="ExternalInput").ap()
    wkT = nc.dram_tensor("wkT", [C, HL * D], f32, kind="ExternalInput").ap()
    wvT = nc.dram_tensor("wvT", [C, HL * D], f32, kind="ExternalInput").ap()
    wpT = nc.dram_tensor("wpT", [HL * D, C], f32, kind="ExternalInput").ap()
    kaug = nc.dram_tensor("kaug", [2, T], f32, kind="ExternalInput").ap()
    qaug = nc.dram_tensor("qaug", [HL, 2, T], f32, kind="ExternalInput").ap()
    vones = nc.dram_tensor("vones", [P, HL], f32, kind="ExternalInput").ap()
    outp = nc.dram_tensor("outp", [T, C], f32, kind="ExternalOutput").ap()

    def mm(out, lhsT, rhs, start, stop):
        nc.tensor.matmul(out, lhsT.bitcast(f32r), rhs.bitcast(f32r),
                         start=start, stop=stop)

    def r(ap):
        # walrus requires every writer of an fp32r-matmul operand to declare
        # fp32r output; the PE truncates to fp22 on read either way.
        return ap.bitcast(f32r)

    with tile.TileContext(nc) as tc, ExitStack() as ctx:
        xp = ctx.enter_context(tc.tile_pool(name="xp", bufs=1))
        wpool = ctx.enter_context(tc.tile_pool(name="wpool", bufs=1))
        vp = ctx.enter_context(tc.tile_pool(name="vp", bufs=1))
        kqp = ctx.enter_context(tc.tile_pool(name="kqp", bufs=2))
        ep = ctx.enter_context(tc.tile_pool(name="ep", bufs=4))
        yp = ctx.enter_context(tc.tile_pool(name="yp", bufs=1))
        mp = ctx.enter_context(tc.tile_pool(name="mp", bufs=2))
        op_pool = ctx.enter_context(tc.tile_pool(name="op", bufs=2))
        ps_mm = ctx.enter_context(tc.tile_pool(name="ps_mm", bufs=2, space="PSUM"))
        ps_s = ctx.enter_context(tc.tile_pool(name="ps_s", bufs=2, space="PSUM"))
        ps_y = ctx.enter_context(tc.tile_pool(name="ps_y", bufs=2, space="PSUM"))

        # ---- resident loads: wq/wk/x interleaved per chunk so the qk
        #      accumulation can start as soon as chunk 0 lands; wv/wp deferred.
        w_sb = {}
        x_sb = []
        for ci in range(CI):
            for nm, srct in (("q", wqT), ("k", wkT)):
                t = wpool.tile([P, HL * D], f32, name=f"w{nm}{ci}", tag=f"w{nm}{ci}")
                nc.sync.dma_start(r(t), r(srct[ci * P:(ci + 1) * P, :]))
                w_sb[nm, ci] = t
            t = xp.tile([P, T], f32, name=f"x{ci}", tag=f"x{ci}")
            nc.sync.dma_start(r(t), r(xT[ci * P:(ci + 1) * P, :]))
            x_sb.append(t)

        yT_sb = [yp.tile([P, T], f32, name=f"yT{i}", tag=f"yT{i}") for i in range(2)]
        denom_sb = [mp.tile([NQ, CS], f32, name=f"denom{i}", tag=f"denom{i}",
                            bufs=1) for i in range(HL)]

        v_sb = []

        def sel(ap, n, base):
            # zero the causally-masked region: keep where free - part + base >= 0
            nc.gpsimd.affine_select(out=r(ap), in_=r(ap), compare_op=GE,
                                    fill=0.0, base=base, pattern=[[1, n]],
                                    channel_multiplier=-1)

        def attention(h):
            qa, ka = qT_a[h], kT_a[h]

            def vsl(kt):
                return v_sb[kt][:, h * (D + 1):(h + 1) * (D + 1)]

            for tq in range(NQ):
                i0 = tq * CS
                psy = ps_y.tile([D + 1, CS], f32, name="psy", tag="y")
                qch = qa[:, i0:i0 + CS]
                # fully-unmasked Tk tiles, two per psum tile, one exp per pair
                for pp in range(2 * tq):
                    ka_, kb_ = 2 * pp, 2 * pp + 1
                    pb = ps_s.tile([P, 2 * CS], f32, name="pb", tag="sbig")
                    mm(pb[:, 0:CS], ka[:, ka_ * P:(ka_ + 1) * P], qch, True, True)
                    mm(pb[:, CS:2 * CS], ka[:, kb_ * P:(kb_ + 1) * P], qch, True, True)
                    eb = ep.tile([P, 2 * CS], f32, name="eb", tag="e")
                    nc.scalar.activation(r(eb), pb, EXP)
                    mm(psy, vsl(ka_), eb[:, 0:CS], start=pp == 0, stop=False)
                    mm(psy, vsl(kb_), eb[:, CS:2 * CS], start=False, stop=False)
                # diagonal tiles d=0..3 (kt = 4tq+d): only i >= j columns are
                # live; pack d0(N=512)+d1(N=384) and d2+d3 (N=256 each).
                k0 = 4 * tq
                pa = ps_s.tile([P, 2 * CS], f32, name="pa", tag="sbig")
                mm(pa[:, 0:CS], ka[:, k0 * P:(k0 + 1) * P], qch, True, True)
                mm(pa[:, CS:CS + 384], ka[:, (k0 + 1) * P:(k0 + 2) * P],
                   qa[:, i0 + P:i0 + CS], True, True)
                ea = ep.tile([P, 2 * CS], f32, name="ea", tag="e")
                nc.scalar.activation(r(ea[:, 0:CS + 384]), pa[:, 0:CS + 384], EXP)
                sel(ea[:, 0:CS], CS, 0)
                sel(ea[:, CS:CS + 384], 384, 0)
                mm(psy, vsl(k0), ea[:, 0:CS], start=tq == 0, stop=False)
                mm(psy[:, P:CS], vsl(k0 + 1), ea[:, CS:CS + 384], False, False)
                pc = ps_s.tile([P, 2 * CS], f32, name="pc", tag="sbig")
                mm(pc[:, 0:256], ka[:, (k0 + 2) * P:(k0 + 3) * P],
                   qa[:, i0 + 256:i0 + CS], True, True)
                mm(pc[:, 256:CS], ka[:, (k0 + 3) * P:(k0 + 4) * P],
                   qa[:, i0 + 256:i0 + CS], True, True)
                ec = ep.tile([P, 2 * CS], f32, name="ec", tag="e")
                nc.scalar.activation(r(ec[:, 0:CS]), pc[:, 0:CS], EXP)
                sel(ec[:, 0:256], 256, 0)
                sel(ec[:, 256:CS], 256, -P)
                mm(psy[:, 256:CS], vsl(k0 + 2), ec[:, 0:256], False, False)
                mm(psy[:, 256:CS], vsl(k0 + 3), ec[:, 256:CS], False, True)
                # denominator row -> denom_sb[h] row tq (partition shift by DMA)
                dstg = mp.tile([D + 1, CS], f32, name="dstg", tag="dstg")
                nc.vector.tensor_copy(dstg[D:D + 1, :], psy[D:D + 1, :])
                nc.sync.dma_start(denom_sb[h][tq:tq + 1, :], dstg[D:D + 1, :])
                # raw y^T rows
                if h % 2 == 0:
                    nc.vector.tensor_copy(r(yT_sb[h // 2][0:D, i0:i0 + CS]),
                                          psy[0:D, :])
                else:
                    ystg = mp.tile([D, CS], f32, name="ystg", tag="ystg")
                    nc.vector.tensor_copy(r(ystg), psy[0:D, :])
                    nc.sync.dma_start(r(yT_sb[h // 2][D:2 * D, i0:i0 + CS]), r(ystg))

        def normalize(h):
            nc.vector.reciprocal(denom_sb[h], denom_sb[h])  # in-place 1/x
            for tq in range(NQ):
                rtmp = mp.tile([1, CS], f32, name="rtmp", tag="rtmp", bufs=1)
                nc.sync.dma_start(rtmp, denom_sb[h][tq:tq + 1, :])
                rb = mp.tile([P, CS], f32, name="rb", tag="rb")
                nc.gpsimd.partition_broadcast(rb, rtmp)
                r0 = (h % 2) * D
                ys = yT_sb[h // 2][r0:r0 + D, tq * CS:(tq + 1) * CS]
                nc.vector.tensor_mul(r(ys), ys, rb[r0:r0 + D, :])

        # ---- qT/kT per head pair, v between, attention + normalize per head --
        qT_a, kT_a = {}, {}

        def qk_pair(m):
            for j in range(2):
                h = 2 * m + j
                qT_a[h] = kqp.tile([D + 2, T], f32, name=f"qTa{h}", tag="qTa")
                kT_a[h] = kqp.tile([D + 2, T], f32, name=f"kTa{h}", tag="kTa")
                nc.sync.dma_start(r(qT_a[h][D:D + 2, :]), r(qaug[h]))
                nc.sync.dma_start(r(kT_a[h][D:D + 2, :]), r(kaug))
            for tq in range(NQ):
                for nm, dst in (("q", qT_a), ("k", kT_a)):
                    ps = ps_mm.tile([P, CS], f32, name=f"ps{nm}", tag="mm")
                    for ci in range(CI):
                        mm(ps, w_sb[nm, ci][:, m * P:(m + 1) * P],
                           x_sb[ci][:, tq * CS:(tq + 1) * CS],
                           start=ci == 0, stop=ci == CI - 1)
                    # even head: direct copy; odd head: stage + DMA shift
                    nc.vector.tensor_copy(
                        r(dst[2 * m][0:D, tq * CS:(tq + 1) * CS]), ps[0:D, :])
                    stg = mp.tile([P, CS], f32, name=f"stg{nm}", tag="stg")
                    nc.vector.tensor_copy(r(stg[D:P, :]), ps[D:P, :])
                    nc.sync.dma_start(
                        r(dst[2 * m + 1][0:D, tq * CS:(tq + 1) * CS]), r(stg[D:P, :]))

        qk_pair(0)

        # ---- v = x @ Wv^T (wv/wp loads deferred behind the qk-critical DMAs)
        for ci in range(CI):
            t = wpool.tile([P, HL * D], f32, name=f"wv{ci}", tag=f"wv{ci}")
            nc.sync.dma_start(r(t), r(wvT[ci * P:(ci + 1) * P, :]))
            w_sb["v", ci] = t
        wp_sb = []
        for i in range(2):
            t = wpool.tile([P, C], f32, name=f"wp{i}", tag=f"wp{i}")
            nc.sync.dma_start(r(t), r(wpT[i * P:(i + 1) * P, :]))
            wp_sb.append(t)
        for tt in range(TT):
            psv = ps_mm.tile([P, HL * D], f32, name="psv", tag="mm")
            for ci in range(CI):
                mm(psv, x_sb[ci][:, tt * P:(tt + 1) * P], w_sb["v", ci],
                   start=ci == 0, stop=ci == CI - 1)
            vt = vp.tile([P, HL * (D + 1)], f32, name=f"v{tt}", tag=f"v{tt}")
            v3 = vt.rearrange("p (h e) -> p h e", h=HL)
            nc.sync.dma_start(r(v3[:, :, D:D + 1]), r(vones.unsqueeze(2)))
            nc.vector.tensor_copy(r(v3[:, :, 0:D]),
                                  psv.rearrange("p (h d) -> p h d", h=HL))
            v_sb.append(vt)

        attention(0)
        normalize(0)
        attention(1)
        normalize(1)
        qk_pair(1)
        attention(2)
        normalize(2)
        attention(3)
        normalize(3)

        # ---- partial projection --------------------------------------------
        for tt in range(TT):
            for nh in range(2):
                psp = ps_mm.tile([P, CS], f32, name="psp", tag="mm")
                for kc in range(2):
                    mm(psp, yT_sb[kc][:, tt * P:(tt + 1) * P],
                       wp_sb[kc][:, nh * CS:(nh + 1) * CS],
                       start=kc == 0, stop=kc == 1)
                ot = op_pool.tile([P, CS], f32, name="ot", tag="o")
                nc.vector.tensor_copy(ot, psp)
                nc.sync.dma_start(outp[tt * P:(tt + 1) * P, nh * CS:(nh + 1) * CS], ot)

    nc.compile()
    _BUILT["nc"] = nc
    return nc


def _prep_inputs(x, w_attn, w_proj):
    """Shard + lay out the full inputs for the 8 cores."""
    x = np.asarray(x, dtype=np.float32)
    w_attn = np.asarray(w_attn, dtype=np.float32)
    w_proj = np.asarray(w_proj, dtype=np.float32)

    slopes = _alibi_slopes(H)
    iota = np.arange(T, dtype=np.float32)
    kaug = np.stack([iota, np.ones(T, np.float32)])  # (2, T)
    xTs = [np.ascontiguousarray(x[b].T) for b in range(B)]

    in_maps = []
    for c in range(N_CORES):
        b, hg = divmod(c, 4)
        r0, r1 = hg * 256, (hg + 1) * 256
        qaug = np.empty((HL, 2, T), np.float32)
        for j in range(HL):
            s = slopes[hg * 4 + j]
            qaug[j, 0, :] = s
            qaug[j, 1, :] = -s * iota
        in_maps.append({
            "xT": xTs[b],
            "wqT": np.ascontiguousarray(w_attn[r0:r1, :].T) * np.float32(0.125),
            "wkT": np.ascontiguousarray(w_attn[C + r0:C + r1, :].T),
            "wvT": np.ascontiguousarray(w_attn[2 * C + r0:2 * C + r1, :].T),
            "wpT": np.ascontiguousarray(w_proj[:, r0:r1].T),
            "kaug": kaug,
            "vones": np.ones((P, HL), np.float32),
            "qaug": qaug,
        })
    return in_maps


def kernel(x, w_attn, w_proj):
    from concourse import bass_utils

    nc = _build()
    in_maps = _prep_inputs(x, w_attn, w_proj)
    res = bass_utils.run_bass_kernel_spmd(nc, in_maps, core_ids=list(range(N_CORES)))
    out = np.zeros((B, T, C), dtype=np.float32)
    for c in range(N_CORES):
        out[c // 4] += res.results[c]["outp"]
    return out
